# revision 9
# baseline (speedup 1.0000x reference)
"""Dark channel prior (15x15 sliding-window min, SAME zero padding) on 8 trn2
cores.

Input [32, 512, 512, 3] f32, output same shape. Pure data parallel: 4 images
per core.

Per-core pipeline (per image, stages pipelined across images by the Tile
framework):
  1. SWDGE (gpsimd) cast-load: f32 DRAM -> bf16 SBUF tile [128, 4, 1536];
     partition p holds rows 4p..4p+3 (one 24KB contiguous descriptor per
     partition -- large descriptors are ~4x faster than the 6KB ones a
     row-major 128-row tile would produce).
  2. Horizontal min tree on DVE in bf16 (2x mode): pixel shifts 1,2,4,7
     (= 3,6,12,21 interleaved-channel elements).
  3. PE transpose (identity matmul) per (row-phase j, col-block c) -> PSUM,
     Act copies PSUM -> V tile whose free dim enumerates rows in order
     (V[p=w', c, q, j] = h[row 4q+j, col 128c+p]).
  4. Vertical min tree on DVE along the free dim.
  5. PE transpose back (stride-4 moving operand), Act copy -> natural-layout
     f32 out tile, one 3MB store per image on the Act HWDGE queue (loads and
     stores live on different queues so they never head-of-line block).

Border outputs (within 7 px of an edge) include the zero padding and the
input is non-negative, so they are exactly 0 -> memset. Interior values are
bf16-rounded (max rel err ~4e-3, well under the 2e-2 gate).
"""

import sys

sys.path.insert(0, "/opt/trn_rl_repo")

import numpy as np

N_CORES = 8
B, H, W, C = 32, 512, 512, 3
WC = W * C  # 1536
IMGS_PER_CORE = B // N_CORES  # 4
ROWS_PER_CORE = IMGS_PER_CORE * H  # 2048
J = 4  # rows per partition per image (512 / 128)

_BUILD_CACHE = {}


def _emit_image(nc, mybir, img, xv, yv, identb, g, dt):
    """Returns stage closures {load_h, fwd, v, back}; tiles are shared
    between stages via the `state` dict."""
    AluOp = mybir.AluOpType
    f32 = mybir.dt.float32
    state = {}

    def load_h():
        _load_h(nc, mybir, img, xv, identb, g, dt, state)

    def fwd():
        _fwd(nc, mybir, img, identb, g, dt, state)

    def v():
        _v(nc, mybir, img, g, dt, state)

    def back():
        _back(nc, mybir, img, yv, identb, g, dt, state)

    return {"load_h": load_h, "fwd": fwd, "v": v, "back": back}


def _load_h(nc, mybir, img, xv, identb, g, dt, state):
    AluOp = mybir.AluOpType

    # ---- cast load: [128, 4, 1536] bf16, partition p = rows 4p..4p+3 ----
    xt = g["xp"].tile([128, J, WC], dt, tag="xt", name=f"xt{img}")
    nc.gpsimd.dma_start(xt[:], xv[img])

    # ---- horizontal min tree (pixel shifts 1, 2, 4, 7) ----
    m2 = g["mp"].tile([128, J, 1533], dt, tag="mA", name=f"m2_{img}")
    nc.vector.tensor_tensor(m2[:], xt[:, :, 0:1533], xt[:, :, 3:1536], AluOp.min)
    m4 = g["mp"].tile([128, J, 1527], dt, tag="mB", name=f"m4_{img}")
    nc.vector.tensor_tensor(m4[:], m2[:, :, 0:1527], m2[:, :, 6:1533], AluOp.min)
    m8 = g["mp"].tile([128, J, 1515], dt, tag="mA", name=f"m8_{img}")
    nc.vector.tensor_tensor(m8[:], m4[:, :, 0:1515], m4[:, :, 12:1527], AluOp.min)
    ht = g["hp"].tile([128, J, WC], dt, tag="ht", name=f"ht{img}")
    # even-width border zeroing first (Act memzero needs uint32-divisible
    # spans); the tree's final op overwrites the overlap cols 21/1514
    nc.scalar.memzero(ht[:, :, 0:22])
    nc.scalar.memzero(ht[:, :, 1514:1536])
    nc.vector.tensor_tensor(
        ht[:, :, 21:1515], m8[:, :, 0:1494], m8[:, :, 21:1515], AluOp.min
    )
    state["ht"] = ht


def _fwd(nc, mybir, img, identb, g, dt, state):
    ht = state["ht"]

    # ---- forward transpose: V[p=w', c, q, j] = h[row 4q+j, col 128c+p] ----
    V = g["vp"].tile([128, 12, 128, J], dt, tag="V", name=f"V{img}")
    for j in range(J):
        for cg in range(3):
            pt = g["ps"].tile([128, 512], dt, tag="ps", name=f"pt{img}_{j}_{cg}")
            for cs in range(4):
                c = 4 * cg + cs
                nc.tensor.transpose(
                    pt[:, 128 * cs : 128 * (cs + 1)],
                    ht[:, j, 128 * c : 128 * (c + 1)],
                    identb[:],
                )
            nc.scalar.copy(
                V[:, 4 * cg : 4 * (cg + 1), :, j],
                pt[:].rearrange("p (a b) -> p a b", a=4),
            )
    state["V"] = V


def _v(nc, mybir, img, g, dt, state):
    AluOp = mybir.AluOpType
    V = state["V"]

    # ---- vertical min tree along free dim (rows in order) ----
    Vv = V[:].rearrange("p c q j -> p c (q j)")  # [128, 12, 512]
    v2 = g["vmp"].tile([128, 12, 511], dt, tag="vA", name=f"v2_{img}")
    nc.vector.tensor_tensor(v2[:], Vv[:, :, 0:511], Vv[:, :, 1:512], AluOp.min)
    v4 = g["vmp"].tile([128, 12, 509], dt, tag="vB", name=f"v4_{img}")
    nc.vector.tensor_tensor(v4[:], v2[:, :, 0:509], v2[:, :, 2:511], AluOp.min)
    v8 = g["vmp"].tile([128, 12, 505], dt, tag="vA", name=f"v8_{img}")
    nc.vector.tensor_tensor(v8[:], v4[:, :, 0:505], v4[:, :, 4:509], AluOp.min)
    Wt = g["wp"].tile([128, 12, 128, J], dt, tag="W", name=f"W{img}")
    Wv = Wt[:].rearrange("p c q j -> p c (q j)")
    nc.scalar.memzero(Wv[:, :, 0:8])
    nc.scalar.memzero(Wv[:, :, 504:512])
    nc.vector.tensor_tensor(
        Wv[:, :, 7:505], v8[:, :, 0:498], v8[:, :, 7:505], AluOp.min
    )
    state["W"] = Wt


def _back(nc, mybir, img, yv, identb, g, dt, state):
    f32 = mybir.dt.float32
    Wt = state["W"]

    # ---- back transpose + natural output tile + store ----
    ot = g["op_"].tile([128, J, WC], f32, tag="ot", name=f"ot{img}")
    for j in range(J):
        for cg in range(3):
            pt2 = g["ps2"].tile(
                [128, 512], dt, tag="ps2", name=f"pt2_{img}_{j}_{cg}"
            )
            for cs in range(4):
                c = 4 * cg + cs
                nc.tensor.transpose(
                    pt2[:, 128 * cs : 128 * (cs + 1)], Wt[:, c, :, j], identb[:]
                )
            nc.scalar.copy(ot[:, j, 512 * cg : 512 * (cg + 1)], pt2[:])
    nc.scalar.dma_start(yv[img], ot[:])


def _build(repeat=1, n_cores=N_CORES, bufs=None):
    key = (repeat, n_cores, tuple(sorted((bufs or {}).items())))
    if key in _BUILD_CACHE:
        return _BUILD_CACHE[key]

    from contextlib import ExitStack, nullcontext

    import concourse.bacc as bacc
    import concourse.tile as tile
    import ml_dtypes
    from concourse import mybir
    from concourse.bass_interp import get_hw_module

    f32 = mybir.dt.float32
    dt = mybir.dt.bfloat16
    BUFS = dict(xp=2, mp=1, hp=2, vp=2, vmp=1, wp=2, op=2, ps=4, ps2=4)
    BUFS.update(bufs or {})

    nc = bacc.Bacc(
        "TRN2", target_bir_lowering=False, debug=False, num_devices=n_cores
    )
    x = nc.dram_tensor("x", [ROWS_PER_CORE, WC], f32, kind="ExternalInput")
    y = nc.dram_tensor("y", [ROWS_PER_CORE, WC], f32, kind="ExternalOutput")
    identb_dram = nc.inline_tensor(
        np.eye(128).astype(ml_dtypes.bfloat16), name="identb"
    )

    xv = x.ap().rearrange("(i p j) w -> i p j w", i=IMGS_PER_CORE, p=128, j=J)
    yv = y.ap().rearrange("(i p j) w -> i p j w", i=IMGS_PER_CORE, p=128, j=J)

    with tile.TileContext(nc) as tc, ExitStack() as ctx:
        cpool = ctx.enter_context(tc.tile_pool(name="const", bufs=1))
        g = dict(
            xp=ctx.enter_context(tc.tile_pool(name="xp", bufs=BUFS["xp"])),
            mp=ctx.enter_context(tc.tile_pool(name="mp", bufs=BUFS["mp"])),
            hp=ctx.enter_context(tc.tile_pool(name="hp", bufs=BUFS["hp"])),
            vp=ctx.enter_context(tc.tile_pool(name="vp", bufs=BUFS["vp"])),
            vmp=ctx.enter_context(tc.tile_pool(name="vmp", bufs=BUFS["vmp"])),
            wp=ctx.enter_context(tc.tile_pool(name="wp", bufs=BUFS["wp"])),
            op_=ctx.enter_context(tc.tile_pool(name="op", bufs=BUFS["op"])),
            ps=ctx.enter_context(
                tc.tile_pool(name="ps", bufs=BUFS["ps"], space="PSUM")
            ),
            ps2=ctx.enter_context(
                tc.tile_pool(name="ps2", bufs=BUFS["ps2"], space="PSUM")
            ),
        )
        identb = cpool.tile([128, 128], dt)
        nc.scalar.dma_start(identb[:], identb_dram.ap())

        loop_cm = tc.For_i(0, repeat, 1) if repeat > 1 else nullcontext()
        with loop_cm:
            # Software-pipelined emission: engines run their queues in
            # program order, so interleave images to keep DVE dense:
            #   DVE: h0 h1 v0 h2 v1 h3 v2 v3
            #   Act: F0 F1 B0 F2 B1 F3 B2 B3 (+stores)
            st = [
                _emit_image(nc, mybir, img, xv, yv, identb, g, dt)
                for img in range(IMGS_PER_CORE)
            ]
            st[0]["load_h"]()
            st[1]["load_h"]()
            st[0]["fwd"]()
            st[0]["v"]()
            st[2]["load_h"]()
            st[1]["fwd"]()
            st[0]["back"]()
            st[1]["v"]()
            st[3]["load_h"]()
            st[2]["fwd"]()
            st[1]["back"]()
            st[2]["v"]()
            st[3]["fwd"]()
            st[2]["back"]()
            st[3]["v"]()
            st[3]["back"]()

    nc.finalize()
    nc.m = get_hw_module(nc.m)
    _BUILD_CACHE[key] = nc
    return nc


def run_sharded(full_input, repeat=1, n_cores=N_CORES, bufs=None, **kw):
    from concourse.bass_utils import run_bass_kernel_spmd

    nc = _build(repeat=repeat, n_cores=n_cores, bufs=bufs)
    xs = np.ascontiguousarray(full_input, dtype=np.float32).reshape(
        n_cores, ROWS_PER_CORE, WC
    )
    in_maps = [{"x": xs[i]} for i in range(n_cores)]
    res = run_bass_kernel_spmd(nc, in_maps, list(range(n_cores)), **kw)
    out = np.stack([res.results[i]["y"] for i in range(n_cores)])
    return out.reshape(B, H, W, C), res


def kernel(inputs: np.ndarray) -> np.ndarray:
    out, _ = run_sharded(np.asarray(inputs))
    return out.astype(np.float32)
